# revision 15
# baseline (speedup 1.0000x reference)
"""MultiHeadAttention Trainium2 kernel (8-core SPMD, head/tensor parallel).

Problem (hardcoded shapes): stream (2048, 2, 1024) f32, mask (1, 2048, 2048),
w_qkv (1024, 3072), b_qkv (3072,), w_out (1024, 1024), b_out (1024,).
N=2048, B=2, HEADS=16, D_KQ=D_V=64, D_HEAD=192.

Sharding: core d handles batch b=d//4 and the 4 heads [4*(d%4), 4*(d%4)+4);
the post-projection all-reduce (sum over the 4 cores of each batch, + b_out)
is done on the host during unsharding.

Everything runs in the "transposed" orientation (no on-device transposes; the
host pre-transposes stream and mask while sharding). The tensor-engine work
rides fp8e4m3 DoubleRow matmuls (2 contraction k-tiles per pass at 0.5
cycles/row) with hi+lo error compensation so precision stays at bf16 level:

  projections  x and w are pre-split on the host into fp8 hi+lo pairs
               (x*8, w*64 so fp8 normals cover the value range); the three
               significant cross terms (hi*hi, lo*hi, hi*lo) give 24
               128-blocks of contraction = 12 DoubleRow passes instead of 8
               fp32r passes. q/k bias is folded into the PSUM->fp8 cast
               (tensor_scalar mult+add, per-partition bias), v bias rides a
               K=1 f32r matmul.
  q/k strips   per head j, qhl[j] = [q_hi(64p) | q_lo(64p)] x N (x16 scale)
               and khl[j] = [k_hi | k_lo] per m-tile, duplicated across both
               partition halves by SBUF->SBUF DMA so one DoubleRow matmul
               per (head, m-tile, 512-block) contracts all four hi/lo cross
               terms: logits*256 at 0.5 cycles/row.
  mask         two forms, chosen per (chunk, m-tile) to balance PE vs DVE:
               DVE-tiles load exp(maskT) bf16 and multiply after the exp
               (2-byte 2x DVE rate); PE-tiles load maskT*16 as fp8 hi+lo and
               add it into the logits PSUM with an identity*16 DoubleRow
               matmul, so the exp needs no follow-up multiply.
  exp          Act engine, scale=1/256 folded in, bf16 out. The Act engine
               (128 exps, ~134us) is the designed critical resource.
  PV           bf16 with lhsT = [v | ones] so the PSUM carries values^T and
               the 64x-replicated softmax denominators Z for free.
  outproj      f32r-free: valT/wout in bf16, one DVE copy per 512 cols,
               f32 DMA to HBM.

Scheduling: proj_v m-tiles 2..15 and proj_head(2)/(3) column blocks run as
fillers inside the first attention phase (staggered so v1[mt] lands before
the mt loop reaches it); outproj(c0) fills attn(1,1); DMA queues are split
by dependency class (sync: mask prefetch ring + strip moves, scalar: initial
loads, vector: zm moves + output writes) and nothing issues DMA from Pool
(software DGE runs on the Pool engine itself).
"""

import numpy as np
import ml_dtypes

import concourse.tile as tile
from concourse import bacc, mybir
from concourse.bass_utils import run_bass_kernel_spmd

BF16 = ml_dtypes.bfloat16
F8 = ml_dtypes.float8_e4m3fn
dt = mybir.dt
AF = mybir.ActivationFunctionType
ALU = mybir.AluOpType
DR = mybir.MatmulPerfMode.DoubleRow

# Shapes (hardcoded per the problem spec)
N = 2048          # sequence length
B = 2             # batch
DSTR = 1024       # d_stream
HEADS = 16        # total heads
NH = 4            # heads per core
DKQ = 64
DV = 64
DHEAD = 2 * DKQ + DV
P = 128
KT = DSTR // P    # 8 contraction k-tiles for projections
MT = N // P       # 16 m-tiles
CH = 1024         # attention n-chunk width
NCH = N // CH     # 2 chunks
NB = 512          # matmul moving free dim
N_CORES = 8


# fp8 scaling: x*XS and w*WS on host so e4m3 normals cover the ranges;
# q/k cast at *QS; the QK logits then carry QS^2 which the exp unscales.
# The PE-added mask is maskT*MS with an identity*MS lhsT (MS*MS = QS*QS).
XS, WS, QS, MS = 8.0, 64.0, 16.0, 16.0
CS = QS / (XS * WS)      # PSUM -> q/k strip scale
ES = 1.0 / (QS * QS)     # exp() input scale
VS = 1.0 / (XS * WS)     # PSUM -> v1 scale

# (c, mt) tiles whose mask is added on the PE (fp8 DR identity) instead of
# multiplied on the DVE. Keep them in late phases where PE has slack.
PE_MASK = {(1, 6), (1, 13)}

f32, f32r, bf16, fp8 = dt.float32, dt.float32r, dt.bfloat16, dt.float8e4

_BUILT = {}


def _build_nc():
    """Build + compile the single-core SPMD Bass program (same on all cores)."""
    nc = bacc.Bacc("TRN2", target_bir_lowering=False, debug=False)

    xhi = nc.dram_tensor("xhi", [DSTR, N], fp8, kind="ExternalInput").ap()
    xlo = nc.dram_tensor("xlo", [DSTR, N], fp8, kind="ExternalInput").ap()
    wqh = nc.dram_tensor("wqh", [DSTR, 4 * P], fp8, kind="ExternalInput").ap()
    wql = nc.dram_tensor("wql", [DSTR, 4 * P], fp8, kind="ExternalInput").ap()
    wvh = nc.dram_tensor("wvh", [DSTR, NH * DV], fp8, kind="ExternalInput").ap()
    wvl = nc.dram_tensor("wvl", [DSTR, NH * DV], fp8, kind="ExternalInput").ap()
    bqk = nc.dram_tensor("bqk", [1, 4 * P], f32r, kind="ExternalInput").ap()
    bv = nc.dram_tensor("bv", [1, NH * DV], f32r, kind="ExternalInput").ap()
    ones = nc.dram_tensor("ones", [1, NB], f32r, kind="ExternalInput").ap()
    id16 = nc.dram_tensor("id16", [P, 2 * P], fp8, kind="ExternalInput").ap()
    emT = nc.dram_tensor("emT", [N, N], bf16, kind="ExternalInput").ap()
    mT8 = nc.dram_tensor("mT8", [N, 2 * N], fp8, kind="ExternalInput").ap()
    wout = nc.dram_tensor("wout", [NH * DV, DSTR], bf16, kind="ExternalInput").ap()
    out = nc.dram_tensor("out", [N, DSTR], f32, kind="ExternalOutput").ap()

    with tile.TileContext(nc) as tc:
        with (
            tc.tile_pool(name="consts", bufs=1) as consts,
            tc.tile_pool(name="xw", bufs=1) as xw_p,
            tc.tile_pool(name="qk", bufs=1) as qk_p,
            tc.tile_pool(name="hl", bufs=2) as hl_p,
            tc.tile_pool(name="v1", bufs=1) as v1_p,
            tc.tile_pool(name="valT", bufs=1) as valT_p,
            tc.tile_pool(name="em", bufs=1) as em_p,
            tc.tile_pool(name="wT", bufs=4) as wT_p,
            tc.tile_pool(name="z", bufs=1) as z_p,
            tc.tile_pool(name="ob", bufs=2) as ob_p,
            tc.tile_pool(name="ps", bufs=1, space="PSUM") as ps_p,
        ):
            # ---------- persistent SBUF ----------
            xhi_sb = xw_p.tile([P, KT, N], fp8)
            xlo_sb = xw_p.tile([P, KT, N], fp8)
            wqh_sb = xw_p.tile([P, KT, 4 * P], fp8)
            wql_sb = xw_p.tile([P, KT, 4 * P], fp8)
            wvh_sb = xw_p.tile([P, KT, NH * DV], fp8)
            wvl_sb = xw_p.tile([P, KT, NH * DV], fp8)
            bqk_sb = consts.tile([1, 4 * P], f32r)
            bv_sb = consts.tile([1, NH * DV], f32r)
            ones1 = consts.tile([1, NB], f32r)
            id16_sb = consts.tile([P, 2, P], fp8)
            wout_sb = consts.tile([P, 2, DSTR], bf16)
            kt_maj = "(k p) n -> p k n"
            nc.scalar.dma_start(out=wqh_sb, in_=wqh.rearrange(kt_maj, p=P))
            nc.scalar.dma_start(out=xhi_sb, in_=xhi.rearrange(kt_maj, p=P))
            nc.scalar.dma_start(out=wql_sb, in_=wql.rearrange(kt_maj, p=P))
            nc.scalar.dma_start(out=xlo_sb, in_=xlo.rearrange(kt_maj, p=P))
            nc.sync.dma_start(out=wvh_sb, in_=wvh.rearrange(kt_maj, p=P))
            nc.sync.dma_start(out=wvl_sb, in_=wvl.rearrange(kt_maj, p=P))
            nc.sync.dma_start(out=bqk_sb, in_=bqk)
            nc.sync.dma_start(out=bv_sb, in_=bv)
            nc.sync.dma_start(out=ones1, in_=ones)
            nc.sync.dma_start(
                out=id16_sb, in_=id16.rearrange("p (i q) -> p i q", i=2))
            nc.sync.dma_start(
                out=wout_sb, in_=wout.rearrange("(i p) d -> p i d", p=P))

            # q strips: [q_hi(64p) | q_lo(64p)] x {2 identical k-tile slots} x N
            # k strips: per m-tile, {k_hi, k_lo} x 128 m, both partition halves
            qhl = [qk_p.tile([P, 2, N], fp8, name=f"qhl{j}") for j in range(NH)]
            khl = [qk_p.tile([P, MT, 2, P], fp8, name=f"khl{j}") for j in range(NH)]

            # v1 lhsT slots per (mt, head): even-in-pair = [v | ones],
            # odd-in-pair = [ones | v] -> PV output carries values rows and
            # 64x-replicated Z rows in complementary partition halves.
            v1 = v1_p.tile([P, MT, NH, P], bf16)
            nc.gpsimd.memset(v1[:, :, 0::2, 64:128], 1.0)
            nc.gpsimd.memset(v1[:, :, 1::2, 0:64], 1.0)
            valT = [valT_p.tile([P, N], bf16, name=f"valT{i}") for i in range(2)]
            # mask ring: bf16 em view and fp8 hi/lo view share the ring slot
            # count but are separate buffers (only one form is loaded per mt).
            em_sb = em_p.tile([P, MT, CH], bf16)
            mhl_sb = em_p.tile([P, len(PE_MASK), 2, CH], fp8)
            mhl_ix = {k: i for i, k in enumerate(sorted(PE_MASK))}

            # 12 DoubleRow term/k-tile pairs for the 3-term hi/lo projection
            def dr_terms(wh, wl):
                for t, (xa, wa) in enumerate(
                        ((xhi_sb, wh), (xlo_sb, wh), (xhi_sb, wl))):
                    for kt in range(0, KT, 2):
                        yield (t == 0 and kt == 0), (t == 2 and kt == KT - 2), \
                            xa, wa, kt

            # ---------- q/k projection + fp8 hi/lo strip build ----------
            # strips[j] = (hi8s, lo8s): per-head [q|k] staging, fp8, *QS scale
            strips = {}

            def proj_head_nb(j, nb, tag):
                if nb == 0:
                    strips[j] = (
                        hl_p.tile([P, 4, NB], fp8, name=f"hi8s{j}"),
                        hl_p.tile([P, 4, NB], fp8, name=f"lo8s{j}"),
                    )
                hi8s, lo8s = strips[j]
                jc = slice(j * P, (j + 1) * P)
                nbc = slice(nb * NB, (nb + 1) * NB)
                ps = ps_p.tile([P, NB], f32, tag=tag)
                for st, sp, xa, wa, kt in dr_terms(wqh_sb, wql_sb):
                    nc.tensor.matmul(
                        ps, lhsT=wa[:, kt:kt + 2, jc], rhs=xa[:, kt:kt + 2, nbc],
                        start=st, stop=False, perf_mode=DR,
                    )
                nc.tensor.matmul(
                    ps, lhsT=bqk_sb[:, jc], rhs=ones1, start=False, stop=True)
                nc.vector.tensor_scalar(
                    out=hi8s[:, nb, :], in0=ps, scalar1=CS, scalar2=None,
                    op0=ALU.mult)
                nc.vector.scalar_tensor_tensor(
                    out=lo8s[:, nb, :], in0=ps, scalar=CS, in1=hi8s[:, nb, :],
                    op0=ALU.mult, op1=ALU.subtract)
                if nb == 3:
                    # distribute: q rows to both k-tile slots of qhl (hi on
                    # partitions 0:64, lo on 64:128), k rows duplicated across
                    # both partition halves of khl slots 0 (hi) and 1 (lo).
                    hq, lq = hi8s[0:64, :, :], lo8s[0:64, :, :]
                    hk, lk = hi8s[64:128, :, :], lo8s[64:128, :, :]
                    for t in range(2):
                        nc.sync.dma_start(out=qhl[j][0:64, t, :], in_=hq)
                        nc.sync.dma_start(out=qhl[j][64:128, t, :], in_=lq)
                        nc.sync.dma_start(out=khl[j][64 * t:64 * t + 64, :, 0, :], in_=hk)
                        nc.sync.dma_start(out=khl[j][64 * t:64 * t + 64, :, 1, :], in_=lk)

            def proj_head(j):
                with nc.named_scope(f"proj_head{j}"):
                    for nb in range(N // NB):
                        proj_head_nb(j, nb, ["A", "B", "V0", "V1"][nb % 4])

            # ---------- v projection (one m-tile) ----------
            def proj_v(mt, tag):
                with nc.named_scope(f"proj_v{mt}"):
                    mc = slice(mt * P, (mt + 1) * P)
                    ps = ps_p.tile([P, NH * DV], f32, tag=tag)
                    for st, sp, xa, wa, kt in dr_terms(wvh_sb, wvl_sb):
                        nc.tensor.matmul(
                            ps, lhsT=xa[:, kt:kt + 2, mc], rhs=wa[:, kt:kt + 2, :],
                            start=st, stop=False, perf_mode=DR,
                        )
                    nc.tensor.matmul(
                        ps, lhsT=ones1[:, 0:P], rhs=bv_sb, start=False, stop=True)
                    psj = ps.rearrange("p (j d) -> p j d", d=DV)
                    nc.vector.tensor_scalar(
                        out=v1[:, mt, 0::2, 0:DV], in0=psj[:, 0::2, :],
                        scalar1=VS, scalar2=None, op0=ALU.mult)
                    nc.vector.tensor_scalar(
                        out=v1[:, mt, 1::2, 64:64 + DV], in0=psj[:, 1::2, :],
                        scalar1=VS, scalar2=None, op0=ALU.mult)

            # ---------- output projection for one n row-tile ----------
            def outproj_nt(nt, tail=False):
                with nc.named_scope(f"outproj{nt}"):
                    for ds in range(DSTR // NB):
                        ps = ps_p.tile([P, NB], f32, tag="AB"[(2 * nt + ds) % 2])
                        for p in range(2):
                            nc.tensor.matmul(
                                ps,
                                lhsT=valT[p][:, nt * P:(nt + 1) * P],
                                rhs=wout_sb[:, p, ds * NB:(ds + 1) * NB],
                                start=(p == 0), stop=(p == 1),
                            )
                        ob = ob_p.tile([P, NB], f32)
                        if tail and ds == 1:
                            nc.scalar.copy(out=ob, in_=ps)
                        else:
                            nc.vector.tensor_copy(out=ob, in_=ps)
                        nc.sync.dma_start(
                            out=out[nt * P:(nt + 1) * P,
                                    ds * NB:(ds + 1) * NB], in_=ob)

            # ---------- attention for one head pair over one n-chunk ----------
            # fillers: dict mt -> list of callables run between the two heads
            def attn(p, c, fillers=()):
                fillers = dict(fillers)
                with nc.named_scope(f"attn_p{p}_c{c}"):
                    psv = [
                        ps_p.tile([P, CH], f32, tag=f"V{oe}",
                                  name=f"psv{oe}_p{p}_c{c}")
                        for oe in (0, 1)
                    ]
                    for mt in range(MT):
                        pe_mask = (c, mt) in PE_MASK
                        if p == 0:
                            if pe_mask:
                                nc.sync.dma_start(
                                    out=mhl_sb[:, mhl_ix[(c, mt)], :, :],
                                    in_=mT8[mt * P:(mt + 1) * P, :].rearrange(
                                        "p (i n) -> p i n", i=2)[
                                        :, :, c * CH:(c + 1) * CH],
                                )
                            if mt % 4 == 0:
                                # batched load of 4 m-tiles of exp(maskT)
                                nc.sync.dma_start(
                                    out=em_sb[:, mt:mt + 4, :],
                                    in_=emT[mt * P:(mt + 4) * P,
                                            c * CH:(c + 1) * CH].rearrange(
                                        "(i p) n -> p i n", p=P),
                                )
                        for oe in (0, 1):
                            j = 2 * p + oe
                            psl = ps_p.tile([P, CH], f32, tag="AB"[oe])
                            for h2 in range(CH // NB):
                                cb = c * CH + h2 * NB
                                nc.tensor.matmul(
                                    psl[:, h2 * NB:(h2 + 1) * NB],
                                    lhsT=khl[j][:, mt, :, :],
                                    rhs=qhl[j][:, :, cb:cb + NB],
                                    start=True, stop=not pe_mask,
                                    perf_mode=DR,
                                )
                                if pe_mask:
                                    nc.tensor.matmul(
                                        psl[:, h2 * NB:(h2 + 1) * NB],
                                        lhsT=id16_sb,
                                        rhs=mhl_sb[:, mhl_ix[(c, mt)], :,
                                                   h2 * NB:(h2 + 1) * NB],
                                        start=False, stop=True, perf_mode=DR,
                                    )
                            wt = wT_p.tile([P, CH], bf16)
                            nc.scalar.activation(out=wt, in_=psl, func=AF.Exp,
                                                 scale=ES)
                            if not pe_mask:
                                pool_mul = (p, c) != (0, 0) and oe == 1 and mt >= 2
                                eng = nc.gpsimd if pool_mul else nc.vector
                                eng.tensor_mul(out=wt, in0=wt, in1=em_sb[:, mt, :])
                            for h2 in range(CH // NB):
                                nc.tensor.matmul(
                                    psv[oe][:, h2 * NB:(h2 + 1) * NB],
                                    lhsT=v1[:, mt, j, :],
                                    rhs=wt[:, h2 * NB:(h2 + 1) * NB],
                                    start=(mt == 0), stop=(mt == MT - 1),
                                )
                            if oe == 0:
                                for fill in fillers.get(mt, ()):
                                    fill()
                    # normalization: recip the replicated Z rows, DMA-move the
                    # reciprocal block to the values rows' partitions, multiply.
                    with nc.named_scope(f"norm_p{p}_c{c}"):
                        for h2 in range(CH // NB):
                            hb = slice(h2 * NB, (h2 + 1) * NB)
                            cs = slice(c * CH + h2 * NB, c * CH + (h2 + 1) * NB)
                            zr = z_p.tile([P, NB], f32, tag="zr")
                            zm = z_p.tile([P, NB], f32, tag="zm")
                            # even head: values rows 0:64, Z rows 64:128
                            nc.vector.reciprocal(
                                out=zr[64:128, :], in_=psv[0][64:128, hb])
                            nc.sync.dma_start(out=zm[0:64, :], in_=zr[64:128, :])
                            nc.vector.tensor_mul(
                                out=valT[p][0:64, cs], in0=psv[0][0:64, hb],
                                in1=zm[0:64, :],
                            )
                            # odd head: Z rows 0:64, values rows 64:128
                            nc.vector.reciprocal(
                                out=zr[0:64, :], in_=psv[1][0:64, hb])
                            nc.sync.dma_start(out=zm[64:128, :], in_=zr[0:64, :])
                            nc.vector.tensor_mul(
                                out=valT[p][64:128, cs], in1=zm[64:128, :],
                                in0=psv[1][64:128, hb],
                            )

            # ---------- schedule ----------
            proj_head(0)
            for slot in range(MT):
                nc.gpsimd.tensor_copy(
                    out=em_sb[:, slot, 0:1], in_=qhl[0][:, 0, 0:1])
            proj_head(1)
            proj_v(0, "V0")
            proj_v(1, "V1")
            # phase 1: pair 0 / chunk 0, with proj_v 2..15 staggered two m-
            # tiles ahead of the consuming loop and proj_head(2)/(3) blocks.
            f1 = {mt: [] for mt in range(MT)}
            for mt in range(14):
                f1[mt].append(lambda m=mt: proj_v(m + 2, "AB"[m % 2]))
            for i, (j, nb) in enumerate(
                    [(2, nb) for nb in range(4)] + [(3, nb) for nb in range(4)]):
                f1[2 * i + 1].append(
                    lambda j=j, nb=nb, t="AB"[i % 2]: proj_head_nb(j, nb, t))
            attn(0, 0, f1)
            attn(1, 0)
            f3 = {4 * i + 2: [lambda n=i: outproj_nt(n)] for i in range(4)}
            attn(0, 1, f3)
            f4 = {4 * i + 2: [lambda n=i + 4: outproj_nt(n)] for i in range(4)}
            attn(1, 1, f4)
            for nt in range(8, MT):
                outproj_nt(nt, tail=True)

    nc.compile()
    return nc


def get_nc():
    if "nc" not in _BUILT:
        _BUILT["nc"] = _build_nc()
    return _BUILT["nc"]


def _shard_inputs(stream, mask, w_qkv, b_qkv, w_out):
    """Build the 8 per-core input maps (host-side layout transforms)."""
    stream = np.asarray(stream, np.float32)
    mask = np.asarray(mask, np.float32)
    w_qkv = np.asarray(w_qkv, np.float32)
    b_qkv = np.asarray(b_qkv, np.float32)
    w_out = np.asarray(w_out, np.float32)

    emT = np.exp(mask[0].T).astype(BF16)      # (N, N) exp of transposed mask
    m16 = mask[0].T * MS
    m16_hi = m16.astype(F8)
    m16_lo = (m16 - m16_hi.astype(np.float32)).astype(F8)
    mT8 = np.ascontiguousarray(
        np.stack([m16_hi, m16_lo], axis=1).reshape(N, 2 * N))
    id16 = np.ascontiguousarray(
        np.tile(np.eye(P, dtype=np.float32) * MS, (1, 2)).astype(F8))

    def hilo(a):
        hi = a.astype(F8)
        lo = (a - hi.astype(np.float32)).astype(F8)
        return np.ascontiguousarray(hi), np.ascontiguousarray(lo)

    xs = [hilo(stream[:, b, :].T * XS) for b in range(B)]

    in_maps = []
    for d in range(N_CORES):
        b = d // 4
        heads = [(d % 4) * 4 + j for j in range(NH)]
        # f-tile j = [q_j(64) | k_j(64)] columns, scaled *WS
        qkc = [np.concatenate(
            [w_qkv[:, h * DHEAD:h * DHEAD + DKQ],
             w_qkv[:, h * DHEAD + DKQ:h * DHEAD + 2 * DKQ]], axis=1)
            for h in heads]
        wqk_hi, wqk_lo = hilo(np.concatenate(qkc, axis=1) * WS)
        vc = [w_qkv[:, h * DHEAD + 2 * DKQ:(h + 1) * DHEAD] for h in heads]
        wv_hi, wv_lo = hilo(np.concatenate(vc, axis=1) * WS)
        # bias row for the K=1 PE bias matmul: [bq_j | bk_j] * XS*WS
        bqk_arr = np.concatenate(
            [np.concatenate([b_qkv[h * DHEAD:h * DHEAD + DKQ],
                             b_qkv[h * DHEAD + DKQ:h * DHEAD + 2 * DKQ]])
             for h in heads])[None, :].astype(np.float32) * (XS * WS)
        bvv = np.concatenate(
            [b_qkv[h * DHEAD + 2 * DKQ:(h + 1) * DHEAD] for h in heads])
        bv_arr = (bvv[None, :] * (XS * WS)).astype(np.float32)
        woutd = np.ascontiguousarray(
            np.concatenate([w_out[h * DV:(h + 1) * DV, :] for h in heads],
                           axis=0)).astype(BF16)
        in_maps.append({
            "xhi": xs[b][0], "xlo": xs[b][1],
            "wqh": wqk_hi, "wql": wqk_lo, "wvh": wv_hi, "wvl": wv_lo,
            "bqk": bqk_arr, "bv": bv_arr,
            "ones": np.ones((1, NB), np.float32), "id16": id16,
            "emT": emT, "mT8": mT8, "wout": woutd,
        })
    return in_maps


def kernel(stream, mask, w_qkv, b_qkv, w_out, b_out):
    nc = get_nc()
    in_maps = _shard_inputs(stream, mask, w_qkv, b_qkv, w_out)
    res = run_bass_kernel_spmd(nc, in_maps, core_ids=list(range(N_CORES)))
    b_out = np.asarray(b_out, np.float32)
    out = np.empty((N, B, DSTR), np.float32)
    for b in range(B):
        acc = res.results[4 * b]["out"].copy()
        for i in range(1, 4):
            acc += res.results[4 * b + i]["out"]
        out[:, b, :] = acc + b_out
    return out


# revision 20
# speedup vs baseline: 1.3032x; 1.3032x over previous
"""MultiHeadAttention Trainium2 kernel (8-core SPMD, head/tensor parallel).

Problem (hardcoded shapes): stream (2048, 2, 1024) f32, mask (1, 2048, 2048),
w_qkv (1024, 3072), b_qkv (3072,), w_out (1024, 1024), b_out (1024,).
N=2048, B=2, HEADS=16, D_KQ=D_V=64, D_HEAD=192.

Sharding (per the b*heads head-parallel hint): core d handles batch b=d//4 and
the 4 heads [4*(d%4), 4*(d%4)+4): w_qkv columns and w_out rows are split per
head group, logits/weights are fully local per core, and the post-projection
all-reduce (sum over the 4 cores of each batch, + b_out) is done on the host
during unsharding.

Per-core compute, all in "transposed" orientation so no on-device transposes
are needed (the host pre-transposes stream and mask while sharding):

  qkT[f, n]   = (w_qkv_local.T @ x_b.T)[f, n] + b     (f = head-pair d dims)
  v[m, dv]    = (x_b @ w_v_local)[m, dv] + b_v        (bias via a K=1 matmul)
  logitsT     = per head: lT[m, n] = sum_d k[m,d] q[n,d]   (2 heads row-packed
                in the PE array: K=64 each at tile_position (0,0)/(64,0))
  wT[m, n]    = exp(lT) * exp(maskT)[m, n]            (unnormalized softmax;
                exp(mask) is precomputed on the host, applied as a bf16
                multiply at DVE 2x rate; no max-subtraction needed at these
                logit magnitudes)
  psv         = PV matmul with lhsT = [v | ones-block] so the output carries
                values^T rows plus 64x-replicated row-sums Z (the softmax
                denominator) in the complementary partition half, for free
  valT[hv, n] = psv_values * recip(Z)                 (recip on DVE; the recip
                block is DMA-moved across partitions; DVE is lane-locked)
  out_partial = valT^T @ w_out_local   -> DMA to HBM, host sums per batch

dtypes: float32r (full-rate fp32) for the projections, QK^T and the output
projection; bf16 only on the exp->mask->PV path where DVE 2x mode needs it.
PSUM is managed as 4 rotating 2-bank slots (A/B for logits+projections+output,
V0/V1 for the two PV accumulators of the active head pair).
"""

import numpy as np
import ml_dtypes

import concourse.tile as tile
from concourse import bacc, mybir
from concourse.bass_utils import run_bass_kernel_spmd

BF16 = ml_dtypes.bfloat16
dt = mybir.dt
AF = mybir.ActivationFunctionType

# Shapes (hardcoded per the problem spec)
N = 2048          # sequence length
B = 2             # batch
DSTR = 1024       # d_stream
HEADS = 16        # total heads
NH = 4            # heads per core
DKQ = 64
DV = 64
DHEAD = 2 * DKQ + DV
P = 128
KT = DSTR // P    # 8 contraction k-tiles for projections
MT = N // P       # 16 m-tiles
CH = 1024         # attention n-chunk width
NCH = N // CH     # 2 chunks
NB = 512          # matmul moving free dim
N_CORES = 8

f32, f32r, bf16 = dt.float32, dt.float32r, dt.bfloat16
fp8 = dt.float8e4
F8 = ml_dtypes.float8_e4m3fn
DR = mybir.MatmulPerfMode.DoubleRow
ALU = mybir.AluOpType
QS = 16.0            # q/k fp8 strip scale
ES = 1.0 / (QS * QS)  # exp unscale

_BUILT = {}


def _build_nc():
    """Build + compile the single-core SPMD Bass program (same on all cores)."""
    nc = bacc.Bacc("TRN2", target_bir_lowering=False, debug=False)

    xT = nc.dram_tensor("xT", [DSTR, N], f32r, kind="ExternalInput").ap()
    wqk = nc.dram_tensor("wqk", [DSTR, 4 * P], f32r, kind="ExternalInput").ap()
    wv = nc.dram_tensor("wv", [DSTR, NH * DV], f32r, kind="ExternalInput").ap()
    bqk = nc.dram_tensor("bqk", [1, 4 * P], f32r, kind="ExternalInput").ap()
    bv = nc.dram_tensor("bv", [1, NH * DV], f32r, kind="ExternalInput").ap()
    ones = nc.dram_tensor("ones", [1, NB], f32r, kind="ExternalInput").ap()
    emT = nc.dram_tensor("emT", [N, N], bf16, kind="ExternalInput").ap()
    wout = nc.dram_tensor("wout", [NH * DV, DSTR], f32r, kind="ExternalInput").ap()
    out = nc.dram_tensor("out", [N, DSTR], f32, kind="ExternalOutput").ap()

    with tile.TileContext(nc) as tc:
        with (
            tc.tile_pool(name="consts", bufs=1) as consts,
            tc.tile_pool(name="xw", bufs=1) as xw_p,
            tc.tile_pool(name="qkT", bufs=1) as qkT_p,
            tc.tile_pool(name="v1", bufs=1) as v1_p,
            tc.tile_pool(name="valT", bufs=1) as valT_p,
            tc.tile_pool(name="mask", bufs=2) as mask_p,
            tc.tile_pool(name="wT", bufs=3) as wT_p,
            tc.tile_pool(name="z", bufs=1) as z_p,
            tc.tile_pool(name="hl", bufs=2) as hl_p,
            tc.tile_pool(name="ps", bufs=1, space="PSUM") as ps_p,
        ):
            # ---------- persistent SBUF ----------
            xT_sb = xw_p.tile([P, KT, N], f32r)
            wqk_sb = xw_p.tile([P, KT, 4 * P], f32r)
            wv_sb = xw_p.tile([P, KT, NH * DV], f32r)
            bqk_sb = consts.tile([1, 4 * P], f32r)
            nc.scalar.dma_start(out=bqk_sb, in_=bqk)
            for kt in range(KT):
                e1 = nc.sync if kt % 2 == 0 else nc.scalar
                e2 = nc.scalar if kt % 2 == 0 else nc.sync
                e1.dma_start(out=xT_sb[:, kt, :], in_=xT[kt * P:(kt + 1) * P, :])
                e2.dma_start(out=wqk_sb[:, kt, :], in_=wqk[kt * P:(kt + 1) * P, :])
                e2.dma_start(out=wv_sb[:, kt, :], in_=wv[kt * P:(kt + 1) * P, :])
            bv_sb = consts.tile([1, NH * DV], f32r)
            nc.sync.dma_start(out=bv_sb, in_=bv)
            ones1 = consts.tile([1, NB], f32r)
            nc.sync.dma_start(out=ones1, in_=ones)
            wout_sb = consts.tile([P, 2, DSTR], f32r)
            nc.sync.dma_start(
                out=wout_sb, in_=wout.rearrange("(i p) d -> p i d", p=P))

            # fp8 q/k strips (hi+lo error-compensated, *QS):
            # qhl[j] = [q_hi(64p) | q_lo(64p)] x N; khl[j] per m-tile holds
            # {k_hi, k_lo} duplicated across both partition halves.
            qhl = [qkT_p.tile([P, N], fp8, name=f"qhl{j}") for j in range(NH)]
            khl = [qkT_p.tile([P, MT, 2, P], fp8, name=f"khl{j}") for j in range(NH)]
            strips = {}
            # v1 lhsT slots per (mt, head): even-in-pair = [v | ones],
            # odd-in-pair = [ones | v] -> PV output carries values rows and
            # 64x-replicated Z rows in complementary partition halves.
            v1 = v1_p.tile([P, MT, NH, P], bf16)
            nc.vector.memset(v1[:, :, 0::2, 64:128], 1.0)
            nc.vector.memset(v1[:, :, 1::2, 0:64], 1.0)
            valT = [valT_p.tile([P, N], f32r, tag=f"valT{i}", name=f"valT{i}")
                    for i in range(2)]

            # ---------- projections ----------
            def proj_qk(ft):
                j = ft
                with nc.named_scope(f"proj_qk{ft}"):
                    if True:
                        strips[j] = (
                            hl_p.tile([P, 4, NB], fp8, name="hi8s", tag="hi8s"),
                            hl_p.tile([P, 4, NB], fp8, name="lo8s", tag="lo8s"),
                        )
                    hi8s, lo8s = strips[j]
                    jc = slice(j * P, (j + 1) * P)
                    for nb in range(N // NB):
                        ps = ps_p.tile([P, NB], f32, tag=["A", "B", "V0", "V1"][nb % 4])
                        for kt in range(KT):
                            nc.tensor.matmul(
                                ps,
                                lhsT=wqk_sb[:, kt, jc],
                                rhs=xT_sb[:, kt, nb * NB:(nb + 1) * NB],
                                start=(kt == 0), stop=False,
                            )
                        nc.tensor.matmul(
                            ps, lhsT=bqk_sb[:, jc], rhs=ones1,
                            start=False, stop=True)
                        nc.vector.tensor_scalar(
                            out=hi8s[:, nb, :], in0=ps, scalar1=QS, scalar2=None,
                            op0=ALU.mult)
                        nc.vector.scalar_tensor_tensor(
                            out=lo8s[:, nb, :], in0=ps, scalar=QS,
                            in1=hi8s[:, nb, :], op0=ALU.mult, op1=ALU.subtract)
                    # distribute into the QK strip layout (6 batched DMAs)
                    hq, lq = hi8s[0:64, :, :], lo8s[0:64, :, :]
                    hk, lk = hi8s[64:128, :, :], lo8s[64:128, :, :]
                    nc.sync.dma_start(out=qhl[j][0:64, :], in_=hq)
                    nc.sync.dma_start(out=qhl[j][64:128, :], in_=lq)
                    for t in range(2):
                        nc.scalar.dma_start(
                            out=khl[j][64 * t:64 * t + 64, :, 0, :], in_=hk)
                        nc.scalar.dma_start(
                            out=khl[j][64 * t:64 * t + 64, :, 1, :], in_=lk)

            def proj_v():
                with nc.named_scope("proj_v"):
                    for mt in range(MT):
                        ps = ps_p.tile([P, NH * DV], f32, tag=["A", "B", "V0", "V1"][mt % 4])
                        for kt in range(KT):
                            nc.tensor.matmul(
                                ps,
                                lhsT=xT_sb[:, kt, mt * P:(mt + 1) * P],
                                rhs=wv_sb[:, kt, :],
                                start=(kt == 0), stop=False,
                            )
                        nc.tensor.matmul(
                            ps, lhsT=ones1[:, 0:P], rhs=bv_sb,
                            start=False, stop=True,
                        )
                        psj = ps.rearrange("p (j d) -> p j d", d=DV)
                        nc.vector.tensor_copy(
                            out=v1[:, mt, 0::2, 0:DV], in_=psj[:, 0::2, :])
                        nc.vector.tensor_copy(
                            out=v1[:, mt, 1::2, 64:64 + DV], in_=psj[:, 1::2, :])

            # ---------- attention for one head pair over one n-chunk ----------
            def attn(p, c):
                with nc.named_scope(f"attn_p{p}_c{c}"):
                    psv = [
                        ps_p.tile([P, CH], f32, tag=f"V{oe}",
                                  name=f"psv{oe}_p{p}_c{c}")
                        for oe in (0, 1)
                    ]
                    for mt in range(MT):
                        em = mask_p.tile([P, CH], bf16)
                        nc.sync.dma_start(
                            out=em,
                            in_=emT[mt * P:(mt + 1) * P, c * CH:(c + 1) * CH],
                        )
                        for oe in (0, 1):
                            j = 2 * p + oe
                            psl = ps_p.tile([P, CH], f32, tag="AB"[oe])
                            for h2 in range(CH // NB):
                                cb = c * CH + h2 * NB
                                nc.tensor.matmul(
                                    psl[:, h2 * NB:(h2 + 1) * NB],
                                    lhsT=khl[j][:, mt, :, :],
                                    rhs=qhl[j][:, cb:cb + NB].unsqueeze(
                                        1).broadcast_to((P, 2, NB)),
                                    start=True, stop=True, perf_mode=DR,
                                )
                            wt = wT_p.tile([P, CH], bf16, bufs=3)
                            nc.scalar.activation(out=wt, in_=psl, func=AF.Exp,
                                                 scale=ES)
                            nc.vector.tensor_mul(out=wt, in0=wt, in1=em)
                            for h2 in range(CH // NB):
                                nc.tensor.matmul(
                                    psv[oe][:, h2 * NB:(h2 + 1) * NB],
                                    lhsT=v1[:, mt, j, :],
                                    rhs=wt[:, h2 * NB:(h2 + 1) * NB],
                                    start=(mt == 0), stop=(mt == MT - 1),
                                )
                    # normalization: recip the replicated Z rows, DMA-move the
                    # reciprocal block to the values rows' partitions, multiply.
                    with nc.named_scope(f"norm_p{p}_c{c}"):
                        zr = z_p.tile([P, CH], f32, tag="zr")
                        zm = z_p.tile([P, CH], f32, tag="zm")
                        cs = slice(c * CH, (c + 1) * CH)
                        # even head: values rows 0:64, Z rows 64:128
                        nc.vector.reciprocal(out=zr[64:128, :], in_=psv[0][64:128, :])
                        nc.sync.dma_start(out=zm[0:64, :], in_=zr[64:128, :])
                        nc.vector.tensor_mul(
                            out=valT[p][0:64, cs], in0=psv[0][0:64, :],
                            in1=zm[0:64, :],
                        )
                        # odd head: Z rows 0:64, values rows 64:128
                        nc.vector.reciprocal(out=zr[0:64, :], in_=psv[1][0:64, :])
                        nc.sync.dma_start(out=zm[64:128, :], in_=zr[0:64, :])
                        nc.vector.tensor_mul(
                            out=valT[p][64:128, cs], in0=psv[1][64:128, :],
                            in1=zm[64:128, :],
                        )

            # ---------- output projection ----------
            def outproj():
                with nc.named_scope("outproj"):
                    for nt2 in range(MT // 2):
                        ob = wT_p.tile([P, 2, DSTR], f32, tag="outbuf")
                        for half in range(2):
                            nt = 2 * nt2 + half
                            for ds in range(DSTR // NB):
                                ps = ps_p.tile([P, NB], f32,
                                               tag="AB"[(2 * nt + ds) % 2])
                                for p in range(2):
                                    nc.tensor.matmul(
                                        ps,
                                        lhsT=valT[p][:, nt * P:(nt + 1) * P],
                                        rhs=wout_sb[:, p, ds * NB:(ds + 1) * NB],
                                        start=(p == 0), stop=(p == 1),
                                    )
                                obs = ob[:, half, ds * NB:(ds + 1) * NB]
                                if ds == 0:
                                    nc.scalar.copy(out=obs, in_=ps)
                                else:
                                    nc.vector.tensor_copy(out=obs, in_=ps)
                        eng = nc.sync
                        eng.dma_start(
                            out=out[nt2 * 2 * P:(nt2 + 1) * 2 * P, :].rearrange(
                                "(h p) d -> p h d", p=P),
                            in_=ob)

            proj_qk(0)
            proj_qk(1)
            proj_v()
            for c in range(NCH):
                attn(0, c)
            proj_qk(2)
            proj_qk(3)
            for c in range(NCH):
                attn(1, c)
            outproj()

    nc.compile()
    return nc


def get_nc():
    if "nc" not in _BUILT:
        _BUILT["nc"] = _build_nc()
    return _BUILT["nc"]


def _shard_inputs(stream, mask, w_qkv, b_qkv, w_out):
    """Build the 8 per-core input maps (host-side layout transforms)."""
    stream = np.asarray(stream, np.float32)
    mask = np.asarray(mask, np.float32)
    w_qkv = np.asarray(w_qkv, np.float32)
    b_qkv = np.asarray(b_qkv, np.float32)
    w_out = np.asarray(w_out, np.float32)

    emT = np.exp(mask[0].T).astype(BF16)  # (N, N) exp of transposed mask
    xT = [np.ascontiguousarray(stream[:, b, :].T) for b in range(B)]

    in_maps = []
    for d in range(N_CORES):
        b = d // 4
        heads = [(d % 4) * 4 + j for j in range(NH)]
        qc = [w_qkv[:, h * DHEAD:h * DHEAD + DKQ] for h in heads]
        kc = [w_qkv[:, h * DHEAD + DKQ:h * DHEAD + 2 * DKQ] for h in heads]
        vc = [w_qkv[:, h * DHEAD + 2 * DKQ:(h + 1) * DHEAD] for h in heads]
        wqk = np.ascontiguousarray(np.concatenate(
            [np.concatenate([qc[i], kc[i]], axis=1) for i in range(NH)], axis=1))
        wv = np.ascontiguousarray(np.concatenate(vc, axis=1))
        bq = [b_qkv[h * DHEAD:h * DHEAD + DKQ] for h in heads]
        bk = [b_qkv[h * DHEAD + DKQ:h * DHEAD + 2 * DKQ] for h in heads]
        bvv = [b_qkv[h * DHEAD + 2 * DKQ:(h + 1) * DHEAD] for h in heads]
        bqk_arr = np.concatenate(
            [np.concatenate([bq[i], bk[i]]) for i in range(NH)]
        )[None, :].astype(np.float32)
        bv_arr = np.ascontiguousarray(np.concatenate(bvv)[None, :])
        woutd = np.ascontiguousarray(
            np.concatenate([w_out[h * DV:(h + 1) * DV, :] for h in heads], axis=0))
        in_maps.append({
            "xT": xT[b], "wqk": wqk, "wv": wv, "bqk": bqk_arr, "bv": bv_arr,
            "ones": np.ones((1, NB), np.float32), "emT": emT, "wout": woutd,
        })
    return in_maps


def kernel(stream, mask, w_qkv, b_qkv, w_out, b_out):
    nc = get_nc()
    in_maps = _shard_inputs(stream, mask, w_qkv, b_qkv, w_out)
    res = run_bass_kernel_spmd(nc, in_maps, core_ids=list(range(N_CORES)))
    b_out = np.asarray(b_out, np.float32)
    out = np.empty((N, B, DSTR), np.float32)
    for b in range(B):
        acc = res.results[4 * b]["out"].copy()
        for i in range(1, 4):
            acc += res.results[4 * b + i]["out"]
        out[:, b, :] = acc + b_out
    return out



# revision 21
# speedup vs baseline: 1.3868x; 1.0641x over previous
"""MultiHeadAttention Trainium2 kernel (8-core SPMD, head/tensor parallel).

Problem (hardcoded shapes): stream (2048, 2, 1024) f32, mask (1, 2048, 2048),
w_qkv (1024, 3072), b_qkv (3072,), w_out (1024, 1024), b_out (1024,).
N=2048, B=2, HEADS=16, D_KQ=D_V=64, D_HEAD=192.

Sharding (per the b*heads head-parallel hint): core d handles batch b=d//4 and
the 4 heads [4*(d%4), 4*(d%4)+4): w_qkv columns and w_out rows are split per
head group, logits/weights are fully local per core, and the post-projection
all-reduce (sum over the 4 cores of each batch, + b_out) is done on the host
during unsharding.

Per-core compute, all in "transposed" orientation so no on-device transposes
are needed (the host pre-transposes stream and mask while sharding):

  qkT[f, n]   = (w_qkv_local.T @ x_b.T)[f, n] + b     (f = head-pair d dims)
  v[m, dv]    = (x_b @ w_v_local)[m, dv] + b_v        (bias via a K=1 matmul)
  logitsT     = per head: lT[m, n] = sum_d k[m,d] q[n,d]   (2 heads row-packed
                in the PE array: K=64 each at tile_position (0,0)/(64,0))
  wT[m, n]    = exp(lT) * exp(maskT)[m, n]            (unnormalized softmax;
                exp(mask) is precomputed on the host, applied as a bf16
                multiply at DVE 2x rate; no max-subtraction needed at these
                logit magnitudes)
  psv         = PV matmul with lhsT = [v | ones-block] so the output carries
                values^T rows plus 64x-replicated row-sums Z (the softmax
                denominator) in the complementary partition half, for free
  valT[hv, n] = psv_values * recip(Z)                 (recip on DVE; the recip
                block is DMA-moved across partitions; DVE is lane-locked)
  out_partial = valT^T @ w_out_local   -> DMA to HBM, host sums per batch

dtypes: float32r (full-rate fp32) for the projections, QK^T and the output
projection; bf16 only on the exp->mask->PV path where DVE 2x mode needs it.
PSUM is managed as 4 rotating 2-bank slots (A/B for logits+projections+output,
V0/V1 for the two PV accumulators of the active head pair).
"""

import numpy as np
import ml_dtypes

import concourse.tile as tile
from concourse import bacc, mybir
from concourse.bass_utils import run_bass_kernel_spmd

BF16 = ml_dtypes.bfloat16
dt = mybir.dt
AF = mybir.ActivationFunctionType

# Shapes (hardcoded per the problem spec)
N = 2048          # sequence length
B = 2             # batch
DSTR = 1024       # d_stream
HEADS = 16        # total heads
NH = 4            # heads per core
DKQ = 64
DV = 64
DHEAD = 2 * DKQ + DV
P = 128
KT = DSTR // P    # 8 contraction k-tiles for projections
MT = N // P       # 16 m-tiles
CH = 1024         # attention n-chunk width
NCH = N // CH     # 2 chunks
NB = 512          # matmul moving free dim
N_CORES = 8

f32, f32r, bf16 = dt.float32, dt.float32r, dt.bfloat16

_BUILT = {}


def _build_nc():
    """Build + compile the single-core SPMD Bass program (same on all cores)."""
    nc = bacc.Bacc("TRN2", target_bir_lowering=False, debug=False)

    xT = nc.dram_tensor("xT", [DSTR, N], f32r, kind="ExternalInput").ap()
    wqk = nc.dram_tensor("wqk", [DSTR, 4 * P], f32r, kind="ExternalInput").ap()
    wv = nc.dram_tensor("wv", [DSTR, NH * DV], f32r, kind="ExternalInput").ap()
    bqk = nc.dram_tensor("bqk", [P, 4], f32, kind="ExternalInput").ap()
    bv = nc.dram_tensor("bv", [1, NH * DV], f32r, kind="ExternalInput").ap()
    ones = nc.dram_tensor("ones", [1, P], f32r, kind="ExternalInput").ap()
    emT = nc.dram_tensor("emT", [N, N], bf16, kind="ExternalInput").ap()
    wout = nc.dram_tensor("wout", [NH * DV, DSTR], f32r, kind="ExternalInput").ap()
    out = nc.dram_tensor("out", [N, DSTR], f32, kind="ExternalOutput").ap()

    with tile.TileContext(nc) as tc:
        with (
            tc.tile_pool(name="consts", bufs=1) as consts,
            tc.tile_pool(name="xw", bufs=1) as xw_p,
            tc.tile_pool(name="qkT", bufs=1) as qkT_p,
            tc.tile_pool(name="v1", bufs=1) as v1_p,
            tc.tile_pool(name="valT", bufs=1) as valT_p,
            tc.tile_pool(name="mask", bufs=3) as mask_p,
            tc.tile_pool(name="wT", bufs=3) as wT_p,
            tc.tile_pool(name="z", bufs=1) as z_p,
            tc.tile_pool(name="ps", bufs=1, space="PSUM") as ps_p,
        ):
            # ---------- persistent SBUF ----------
            xT_sb = xw_p.tile([P, KT, N], f32r)
            wqk_sb = xw_p.tile([P, KT, 4 * P], f32r)
            wv_sb = xw_p.tile([P, KT, NH * DV], f32r)
            bqk_sb = consts.tile([P, 4], f32)
            nc.scalar.dma_start(out=bqk_sb, in_=bqk)
            for kt in range(KT):
                e1 = nc.sync if kt % 2 == 0 else nc.scalar
                e2 = nc.scalar if kt % 2 == 0 else nc.sync
                e1.dma_start(out=xT_sb[:, kt, :], in_=xT[kt * P:(kt + 1) * P, :])
                e2.dma_start(out=wqk_sb[:, kt, :], in_=wqk[kt * P:(kt + 1) * P, :])
                e2.dma_start(out=wv_sb[:, kt, :], in_=wv[kt * P:(kt + 1) * P, :])
            bv_sb = consts.tile([1, NH * DV], f32r)
            nc.sync.dma_start(out=bv_sb, in_=bv)
            ones1 = consts.tile([1, P], f32r)
            nc.sync.dma_start(out=ones1, in_=ones)
            wout_sb = consts.tile([P, 2, DSTR], f32r)
            nc.sync.dma_start(
                out=wout_sb, in_=wout.rearrange("(i p) d -> p i d", p=P))

            # qkT f-tiles: 0 = q pair0, 1 = q pair1, 2 = k pair0, 3 = k pair1
            # (within a tile: partitions 0:64 = even head's d, 64:128 = odd head's)
            qkT = qkT_p.tile([P, 4, N], f32r)
            # v1 lhsT slots per (mt, head): even-in-pair = [v | ones],
            # odd-in-pair = [ones | v] -> PV output carries values rows and
            # 64x-replicated Z rows in complementary partition halves.
            v1 = v1_p.tile([P, MT, NH, P], bf16)
            nc.vector.memset(v1[:, :, 0::2, 64:128], 1.0)
            nc.vector.memset(v1[:, :, 1::2, 0:64], 1.0)
            valT = [valT_p.tile([P, N], f32r, tag=f"valT{i}", name=f"valT{i}")
                    for i in range(2)]

            # ---------- projections ----------
            def proj_qk(ft):
                with nc.named_scope(f"proj_qk{ft}"):
                    for nb in range(N // NB):
                        ps = ps_p.tile([P, NB], f32, tag=["A", "B", "V0", "V1"][nb % 4])
                        for kt in range(KT):
                            nc.tensor.matmul(
                                ps,
                                lhsT=wqk_sb[:, kt, ft * P:(ft + 1) * P],
                                rhs=xT_sb[:, kt, nb * NB:(nb + 1) * NB],
                                start=(kt == 0), stop=(kt == KT - 1),
                            )
                        nc.scalar.activation(
                            out=qkT[:, ft, nb * NB:(nb + 1) * NB], in_=ps,
                            func=AF.Identity, bias=bqk_sb[:, ft:ft + 1],
                        )

            def proj_v():
                with nc.named_scope("proj_v"):
                    for mt in range(MT):
                        ps = ps_p.tile([P, NH * DV], f32, tag=["A", "B", "V0", "V1"][mt % 4])
                        for kt in range(KT):
                            nc.tensor.matmul(
                                ps,
                                lhsT=xT_sb[:, kt, mt * P:(mt + 1) * P],
                                rhs=wv_sb[:, kt, :],
                                start=(kt == 0), stop=False,
                            )
                        nc.tensor.matmul(
                            ps, lhsT=ones1, rhs=bv_sb,
                            start=False, stop=True,
                        )
                        psj = ps.rearrange("p (j d) -> p j d", d=DV)
                        nc.vector.tensor_copy(
                            out=v1[:, mt, 0::2, 0:DV], in_=psj[:, 0::2, :])
                        nc.vector.tensor_copy(
                            out=v1[:, mt, 1::2, 64:64 + DV], in_=psj[:, 1::2, :])

            # ---------- attention for one head pair over one n-chunk ----------
            def attn(p, c):
                with nc.named_scope(f"attn_p{p}_c{c}"):
                    psv = [
                        ps_p.tile([P, CH], f32, tag=f"V{oe}",
                                  name=f"psv{oe}_p{p}_c{c}")
                        for oe in (0, 1)
                    ]
                    for mt in range(MT):
                        em = mask_p.tile([P, CH], bf16)
                        nc.sync.dma_start(
                            out=em,
                            in_=emT[mt * P:(mt + 1) * P, c * CH:(c + 1) * CH],
                        )
                        for oe in (0, 1):
                            j = 2 * p + oe
                            base = oe * 64
                            psl = ps_p.tile([P, CH], f32, tag="AB"[oe])
                            for h2 in range(CH // NB):
                                nc.tensor.matmul(
                                    psl[:, h2 * NB:(h2 + 1) * NB],
                                    lhsT=qkT[base:base + 64, 2 + p,
                                                mt * P:(mt + 1) * P],
                                    rhs=qkT[base:base + 64, p,
                                               c * CH + h2 * NB:
                                               c * CH + (h2 + 1) * NB],
                                    start=True, stop=True,
                                )
                            wt = wT_p.tile([P, CH], bf16, bufs=4)
                            nc.scalar.activation(out=wt, in_=psl, func=AF.Exp)
                            nc.vector.tensor_mul(out=wt, in0=wt, in1=em)
                            for h2 in range(CH // NB):
                                nc.tensor.matmul(
                                    psv[oe][:, h2 * NB:(h2 + 1) * NB],
                                    lhsT=v1[:, mt, j, :],
                                    rhs=wt[:, h2 * NB:(h2 + 1) * NB],
                                    start=(mt == 0), stop=(mt == MT - 1),
                                )
                    # normalization: recip the replicated Z rows, DMA-move the
                    # reciprocal block to the values rows' partitions, multiply.
                    with nc.named_scope(f"norm_p{p}_c{c}"):
                        zr = z_p.tile([P, CH], f32, tag="zr")
                        zm = z_p.tile([P, CH], f32, tag="zm")
                        cs = slice(c * CH, (c + 1) * CH)
                        # even head: values rows 0:64, Z rows 64:128
                        nc.vector.reciprocal(out=zr[64:128, :], in_=psv[0][64:128, :])
                        nc.sync.dma_start(out=zm[0:64, :], in_=zr[64:128, :])
                        nc.vector.tensor_mul(
                            out=valT[p][0:64, cs], in0=psv[0][0:64, :],
                            in1=zm[0:64, :],
                        )
                        # odd head: Z rows 0:64, values rows 64:128
                        nc.vector.reciprocal(out=zr[0:64, :], in_=psv[1][0:64, :])
                        nc.sync.dma_start(out=zm[64:128, :], in_=zr[0:64, :])
                        nc.vector.tensor_mul(
                            out=valT[p][64:128, cs], in0=psv[1][64:128, :],
                            in1=zm[64:128, :],
                        )

            # ---------- output projection ----------
            def outproj():
                with nc.named_scope("outproj"):
                    for nt2 in range(MT // 2):
                        ob = wT_p.tile([P, 2, DSTR], f32, tag="outbuf")
                        for half in range(2):
                            nt = 2 * nt2 + half
                            for ds in range(DSTR // NB):
                                ps = ps_p.tile([P, NB], f32,
                                               tag="AB"[(2 * nt + ds) % 2])
                                for p in range(2):
                                    nc.tensor.matmul(
                                        ps,
                                        lhsT=valT[p][:, nt * P:(nt + 1) * P],
                                        rhs=wout_sb[:, p, ds * NB:(ds + 1) * NB],
                                        start=(p == 0), stop=(p == 1),
                                    )
                                obs = ob[:, half, ds * NB:(ds + 1) * NB]
                                if ds == 0:
                                    nc.scalar.copy(out=obs, in_=ps)
                                else:
                                    nc.vector.tensor_copy(out=obs, in_=ps)
                        eng = nc.sync
                        eng.dma_start(
                            out=out[nt2 * 2 * P:(nt2 + 1) * 2 * P, :].rearrange(
                                "(h p) d -> p h d", p=P),
                            in_=ob)

            proj_qk(0)
            proj_qk(2)
            proj_v()
            for c in range(NCH):
                attn(0, c)
            proj_qk(1)
            proj_qk(3)
            for c in range(NCH):
                attn(1, c)
            outproj()

    nc.compile()
    return nc


def get_nc():
    if "nc" not in _BUILT:
        _BUILT["nc"] = _build_nc()
    return _BUILT["nc"]


def _shard_inputs(stream, mask, w_qkv, b_qkv, w_out):
    """Build the 8 per-core input maps (host-side layout transforms)."""
    stream = np.asarray(stream, np.float32)
    mask = np.asarray(mask, np.float32)
    w_qkv = np.asarray(w_qkv, np.float32)
    b_qkv = np.asarray(b_qkv, np.float32)
    w_out = np.asarray(w_out, np.float32)

    emT = np.exp(mask[0].T).astype(BF16)  # (N, N) exp of transposed mask
    xT = [np.ascontiguousarray(stream[:, b, :].T) for b in range(B)]

    in_maps = []
    for d in range(N_CORES):
        b = d // 4
        heads = [(d % 4) * 4 + j for j in range(NH)]
        qc = [w_qkv[:, h * DHEAD:h * DHEAD + DKQ] for h in heads]
        kc = [w_qkv[:, h * DHEAD + DKQ:h * DHEAD + 2 * DKQ] for h in heads]
        vc = [w_qkv[:, h * DHEAD + 2 * DKQ:(h + 1) * DHEAD] for h in heads]
        wqk = np.ascontiguousarray(np.concatenate(
            [qc[0], qc[1], qc[2], qc[3], kc[0], kc[1], kc[2], kc[3]], axis=1))
        wv = np.ascontiguousarray(np.concatenate(vc, axis=1))
        bq = [b_qkv[h * DHEAD:h * DHEAD + DKQ] for h in heads]
        bk = [b_qkv[h * DHEAD + DKQ:h * DHEAD + 2 * DKQ] for h in heads]
        bvv = [b_qkv[h * DHEAD + 2 * DKQ:(h + 1) * DHEAD] for h in heads]
        bqk_arr = np.stack(
            [np.concatenate([bq[0], bq[1]]), np.concatenate([bq[2], bq[3]]),
             np.concatenate([bk[0], bk[1]]), np.concatenate([bk[2], bk[3]])],
            axis=1).astype(np.float32)
        bv_arr = np.ascontiguousarray(np.concatenate(bvv)[None, :])
        woutd = np.ascontiguousarray(
            np.concatenate([w_out[h * DV:(h + 1) * DV, :] for h in heads], axis=0))
        in_maps.append({
            "xT": xT[b], "wqk": wqk, "wv": wv, "bqk": bqk_arr, "bv": bv_arr,
            "ones": np.ones((1, P), np.float32), "emT": emT, "wout": woutd,
        })
    return in_maps


def kernel(stream, mask, w_qkv, b_qkv, w_out, b_out):
    nc = get_nc()
    in_maps = _shard_inputs(stream, mask, w_qkv, b_qkv, w_out)
    res = run_bass_kernel_spmd(nc, in_maps, core_ids=list(range(N_CORES)))
    b_out = np.asarray(b_out, np.float32)
    out = np.empty((N, B, DSTR), np.float32)
    for b in range(B):
        acc = res.results[4 * b]["out"].copy()
        for i in range(1, 4):
            acc += res.results[4 * b + i]["out"]
        out[:, b, :] = acc + b_out
    return out

